# revision 6
# baseline (speedup 1.0000x reference)
"""Trainium2 Bass kernel for nn_KAN_63230508532179 (dense_mlp).

Model (per reference):
  h = gelu(x[:,:,None] * bw1 + bb1)            # [B,1000,16]
  f = tanh(einsum('bnh,noh->bno', h, bw2)+bb2) # [B,1000,8]
  z = f.reshape(B, 8000)
  z = gelu(z @ wc1.T + bc1)                    # [B,256]
  z = gelu(z @ wc2.T + bc2)                    # [B,128]
  y = z @ wc3.T + bc3                          # [B,300]

Sharding: BRANCH-parallel across the 8 cores — core c owns 126 of the
1000 branches over the FULL batch (so the big combiner weight wc1 is
split 8 ways, minimizing host->device wire bytes, and branch params
stay tiny).  Per-core partial z1 sums are combined with an on-device
ReduceScatter (bf16) that hands core c the batch slice
[512c, 512c+512), on which it runs the small combiner tail.

Device mapping per core:
 - x rows (branches) are replicated 16x into an SBUF buffer xh_all
   [128=(j,k), group, batch] by stride-0 DMA (no matmul needed for the
   branch-layer-1 broadcast).
 - bw1/bb1 are applied as per-partition scale/bias in a fused DVE
   tensor_scalar; gelu runs on [128, 2048] tiles (2 group-pairs).
 - branch layer 2 is a block-diagonal matmul per group (K=128, M=64);
   tanh with per-partition bias AP.
 - wc1 K-slices accumulate z1 [256, 512] in PSUM per batch chunk.
"""

import os
import sys
from contextlib import ExitStack

sys.path.insert(0, "/opt/trn_rl_repo")
os.environ.setdefault("MYCRO_LOCAL_CACHE", "1")

import numpy as np
import ml_dtypes

import concourse.bass as bass
import concourse.tile as tile
from concourse import bacc, mybir
from concourse.bass_utils import run_bass_kernel_spmd

BF16 = mybir.dt.bfloat16
F32 = mybir.dt.float32
F16 = mybir.dt.float16
NPBF16 = ml_dtypes.bfloat16

B, N, H1, H2 = 4096, 1000, 16, 8
C1, C2, OUT = 256, 128, 300
NCORES = 8
NBR = 126                 # real branch slots per core (last core: 118)
NBP = 128                 # padded branch slots per core
NG = 16                   # groups of 8 branches per core
NPAIR = 8                 # group pairs (= wc1 K-chunks of 128)
BCH = 8                   # batch chunks
BC = 512                  # batch per chunk (= per-core tail batch)

_CACHE = {}


def _build_program():
    if "nc" in _CACHE:
        return _CACHE["nc"]

    nc = bacc.Bacc("TRN2", target_bir_lowering=False, debug=False,
                   num_devices=NCORES)

    xt_d = nc.dram_tensor("xt", [128, B], BF16, kind="ExternalInput")
    sc_d = nc.dram_tensor("sc", [128, NG], F32, kind="ExternalInput")
    sb_d = nc.dram_tensor("sb", [128, NG], F32, kind="ExternalInput")
    w2_d = nc.dram_tensor("w2", [128, NG * 64], BF16, kind="ExternalInput")
    b2_d = nc.dram_tensor("b2", [128, NPAIR], F32, kind="ExternalInput")
    wc1_d = nc.dram_tensor("wc1", [128, NPAIR * C1], BF16, kind="ExternalInput")
    wc2_d = nc.dram_tensor("wc2", [128, 256], BF16, kind="ExternalInput")
    bc1_d = nc.dram_tensor("bc1", [128, 2], F32, kind="ExternalInput")
    bc2_d = nc.dram_tensor("bc2", [128, 1], F32, kind="ExternalInput")
    wc3_d = nc.dram_tensor("wc3", [128, OUT], BF16, kind="ExternalInput")
    bc3_d = nc.dram_tensor("bc3", [128, 3], F32, kind="ExternalInput")
    out_d = nc.dram_tensor("out", [OUT, BC], F16, kind="ExternalOutput")

    AF = mybir.ActivationFunctionType
    ALU = mybir.AluOpType

    with ExitStack() as ctx:
        tc = ctx.enter_context(tile.TileContext(nc))
        consts = ctx.enter_context(tc.tile_pool(name="consts", bufs=1))
        h_pool = ctx.enter_context(tc.tile_pool(name="h", bufs=3))
        g_pool = ctx.enter_context(tc.tile_pool(name="g", bufs=3))
        f_pool = ctx.enter_context(tc.tile_pool(name="f", bufs=3))
        z_pool = ctx.enter_context(tc.tile_pool(name="z", bufs=2))
        ps_f = ctx.enter_context(tc.tile_pool(name="psf", bufs=3, space="PSUM"))
        ps_z = ctx.enter_context(tc.tile_pool(name="psz", bufs=2, space="PSUM"))
        dram = ctx.enter_context(tc.tile_pool(name="dram", bufs=1, space="DRAM"))

        # small consts on the scalar HWDGE queue, so they don't queue
        # behind the xh replication DMAs on sync
        def load(d, shape, dt, tag, eng):
            s = consts.tile(shape, dt, tag=tag)
            eng.dma_start(out=s[:], in_=d[:, :])
            return s

        sc_sb = load(sc_d, [128, NG], F32, "sc", nc.scalar)
        sbb_sb = load(sb_d, [128, NG], F32, "sb", nc.scalar)
        w2_sb = load(w2_d, [128, NG * 64], BF16, "w2", nc.scalar)
        b2_sb = load(b2_d, [128, NPAIR], F32, "b2", nc.scalar)
        wc1_sb = load(wc1_d, [128, NPAIR * C1], BF16, "wc1", nc.scalar)
        wc2_sb = load(wc2_d, [128, 256], BF16, "wc2", nc.scalar)
        bc1_sb = load(bc1_d, [128, 2], F32, "bc1", nc.scalar)
        bc2_sb = load(bc2_d, [128, 1], F32, "bc2", nc.scalar)
        wc3_sb = load(wc3_d, [128, OUT], BF16, "wc3", nc.scalar)
        bc3_sb = load(bc3_d, [128, 3], F32, "bc3", nc.scalar)

        # xh_g[16j+k, b] = x[8g+j, b]: stride-0 replication DMAs, one per
        # (g, j) — src row partition-broadcast 16x into a 16-partition slice.
        # Per-group tiles keep the read dependencies fine-grained.
        xh_all = []
        for g in range(NG):
            xh_g = consts.tile([128, B], BF16, tag=f"xh{g}")
            for j in range(8):
                src = xt_d[8 * g + j:8 * g + j + 1, :].partition_broadcast(H1)
                nc.sync.dma_start(out=xh_g[H1 * j:H1 * (j + 1), :], in_=src)
            xh_all.append(xh_g)

        z1_part = dram.tile([BCH * C1, BC], BF16, tag="z1p")   # [2048, 512]
        z1_my = dram.tile([C1, BC], BF16, tag="z1m")

        # ---- main loop: batch chunks x 4-group super-iterations ----
        for nb in range(BCH):
            z_ps = ps_z.tile([128, 1024], F32)
            for tt in range(4):
                hg = h_pool.tile([128, 2048], BF16)
                for q in range(4):
                    g = 4 * tt + q
                    nc.vector.tensor_scalar(
                        out=hg[:, 512 * q:512 * (q + 1)],
                        in0=xh_all[g][:, BC * nb:BC * (nb + 1)],
                        scalar1=sc_sb[:, g:g + 1], scalar2=sbb_sb[:, g:g + 1],
                        op0=ALU.mult, op1=ALU.add)
                hG = g_pool.tile([128, 2048], BF16)
                nc.scalar.activation(hG[:], hg[:], AF.Gelu)
                for p2 in range(2):
                    t = 2 * tt + p2
                    f_ps = ps_f.tile([128, 512], F32)
                    for half in range(2):
                        g = 4 * tt + 2 * p2 + half
                        nc.tensor.matmul(
                            f_ps[64 * half:64 * (half + 1), :],
                            lhsT=w2_sb[:, 64 * g:64 * (g + 1)],
                            rhs=hG[:, 512 * (2 * p2 + half):512 * (2 * p2 + half + 1)],
                            start=True, stop=True)
                    fT = f_pool.tile([128, 512], BF16)
                    nc.scalar.activation(fT[:], f_ps[:], AF.Tanh,
                                         bias=b2_sb[:, t:t + 1], scale=1.0)
                    last = t == NPAIR - 1
                    nc.tensor.matmul(z_ps[:, 0:512],
                                     lhsT=wc1_sb[:, 256 * t:256 * t + 128],
                                     rhs=fT[:], start=(t == 0), stop=last,
                                     skip_group_check=True)
                    nc.tensor.matmul(z_ps[:, 512:1024],
                                     lhsT=wc1_sb[:, 256 * t + 128:256 * t + 256],
                                     rhs=fT[:], start=(t == 0), stop=last,
                                     skip_group_check=True)
            z_sb = z_pool.tile([128, 1024], BF16, tag="z_sb")
            nc.vector.tensor_copy(z_sb[:], z_ps[:])
            nc.sync.dma_start(out=z1_part[256 * nb:256 * nb + 128, :],
                              in_=z_sb[:, 0:512])
            nc.sync.dma_start(out=z1_part[256 * nb + 128:256 * nb + 256, :],
                              in_=z_sb[:, 512:1024])

        # ---- cross-core reduction: core c gets batch chunk c, summed ----
        nc.gpsimd.collective_compute(
            "ReduceScatter", ALU.add,
            replica_groups=[list(range(NCORES))],
            ins=[z1_part.opt()], outs=[z1_my.opt()],
        )

        # ---- combiner tail on this core's 512-row batch slice ----
        z1g = []
        for hk in range(2):
            z1f = z_pool.tile([128, 512], BF16, tag="z1f")
            nc.sync.dma_start(out=z1f[:],
                              in_=z1_my[128 * hk:128 * (hk + 1), :])
            zg = z_pool.tile([128, 512], BF16, tag="z1g")
            nc.scalar.activation(zg[:], z1f[:], AF.Gelu,
                                 bias=bc1_sb[:, hk:hk + 1], scale=1.0)
            z1g.append(zg)

        z2_ps = ps_f.tile([128, 512], F32, tag="f_ps")
        nc.tensor.matmul(z2_ps[:], lhsT=wc2_sb[:, 0:128], rhs=z1g[0][:],
                         start=True, stop=False, skip_group_check=True)
        nc.tensor.matmul(z2_ps[:], lhsT=wc2_sb[:, 128:256], rhs=z1g[1][:],
                         start=False, stop=True, skip_group_check=True)
        z2 = z_pool.tile([128, 512], BF16, tag="z2")
        nc.scalar.activation(z2[:], z2_ps[:], AF.Gelu,
                             bias=bc2_sb[:, 0:1], scale=1.0)

        for i, mrows in ((0, 128), (1, 128), (2, 44)):
            o_ps = ps_f.tile([128, 512], F32, tag="f_ps")
            nc.tensor.matmul(o_ps[0:mrows, :],
                             lhsT=wc3_sb[:, 128 * i:128 * i + mrows],
                             rhs=z2[:], start=True, stop=True)
            o_sb = z_pool.tile([128, 512], F16, tag="o")
            nc.vector.tensor_scalar_add(o_sb[0:mrows, :], o_ps[0:mrows, :],
                                        bc3_sb[0:mrows, i:i + 1])
            nc.sync.dma_start(out=out_d[128 * i:128 * i + mrows, :],
                              in_=o_sb[0:mrows, :])

    nc.compile()
    _CACHE["nc"] = nc
    return nc


def preprocess(x, bw1, bb1, bw2, bb2, wc1, bc1, wc2, bc2, wc3, bc3):
    """Host-side repack of full inputs into per-core input maps."""
    f32 = np.float32
    NPB = NCORES * NBR            # 1008 padded branches

    bw1p = np.zeros((NPB, H1), f32); bw1p[:N] = bw1
    bb1p = np.zeros((NPB, H1), f32); bb1p[:N] = bb1
    bw2p = np.zeros((NPB, H2, H1), f32); bw2p[:N] = bw2
    bb2p = np.zeros((NPB, H2), f32); bb2p[:N] = bb2
    xp = np.zeros((NPB, B), f32); xp[:N] = np.asarray(x).T

    wc1r = np.asarray(wc1).reshape(C1, N, H2)

    # tail constants (replicated on all cores; small)
    wc2_sb = np.ascontiguousarray(
        wc2.T.reshape(2, 128, C2).transpose(1, 0, 2).reshape(128, 256)
    ).astype(NPBF16)
    bc1_sb = np.ascontiguousarray(bc1.reshape(2, 128).T.astype(f32))
    bc2_sb = np.ascontiguousarray(bc2.reshape(C2, 1).astype(f32))
    wc3_sb = np.ascontiguousarray(wc3.T).astype(NPBF16)
    bc3p = np.zeros(384, f32); bc3p[:OUT] = bc3
    bc3_sb = np.ascontiguousarray(bc3p.reshape(3, 128).T)

    in_maps = []
    for c in range(NCORES):
        n0 = c * NBR
        lw1 = np.zeros((NBP, H1), f32); lw1[:NBR] = bw1p[n0:n0 + NBR]
        lb1 = np.zeros((NBP, H1), f32); lb1[:NBR] = bb1p[n0:n0 + NBR]
        lw2 = np.zeros((NBP, H2, H1), f32); lw2[:NBR] = bw2p[n0:n0 + NBR]
        lb2 = np.zeros((NBP, H2), f32); lb2[:NBR] = bb2p[n0:n0 + NBR]
        lx = np.zeros((NBP, B), f32); lx[:NBR] = xp[n0:n0 + NBR]

        # scale/bias [128, NG]: row 16j+k, col g -> lw1[8g+j, k]
        sc = lw1.reshape(NG, 8, H1).transpose(1, 2, 0).reshape(128, NG)
        sbb = lb1.reshape(NG, 8, H1).transpose(1, 2, 0).reshape(128, NG)

        # W2 block-diagonal [128, NG*64]: [16j+k, 64g+8j'+o] nonzero iff j==j'
        W2 = np.zeros((8, H1, NG, 8, H2), f32)      # [j, k, g, j', o]
        lw2g = lw2.reshape(NG, 8, H2, H1)           # [g, j, o, k]
        for j in range(8):
            W2[j, :, :, j, :] = lw2g[:, j].transpose(2, 0, 1)   # [k, g, o]
        w2_sb = W2.reshape(128, NG * 64).astype(NPBF16)

        # b2 [128, NPAIR]: row 64h+8j+o, col t -> lb2[16t+8h+j, o]
        b2_sb = np.ascontiguousarray(
            lb2.reshape(NPAIR, 2, 8, H2).transpose(1, 2, 3, 0).reshape(128, NPAIR))

        # wc1 K-slice [128, NPAIR*C1]: row 64h+8j+o of chunk t,
        # col 256t+mm -> wc1[mm, 8*(n0+16t+8h+j)+o]; zero rows for pads
        wc1l = np.zeros((NBP, H2, C1), f32)         # [local branch, o, mm]
        gidx = n0 + np.arange(NBP)
        valid = (np.arange(NBP) < NBR) & (gidx < N)
        vi = np.where(valid)[0]
        wc1l[vi] = wc1r[:, gidx[vi], :].transpose(1, 2, 0)
        wc1_sb = np.ascontiguousarray(
            wc1l.reshape(NPAIR, 2, 8, H2, C1)
            .transpose(1, 2, 3, 0, 4).reshape(128, NPAIR * C1)).astype(NPBF16)

        m = {
            "xt": np.ascontiguousarray(lx.astype(NPBF16)),
            "sc": np.ascontiguousarray(sc), "sb": np.ascontiguousarray(sbb),
            "w2": np.ascontiguousarray(w2_sb), "b2": b2_sb, "wc1": wc1_sb,
            "wc2": wc2_sb, "bc1": bc1_sb, "bc2": bc2_sb,
            "wc3": wc3_sb, "bc3": bc3_sb,
        }
        in_maps.append(m)
    return in_maps


def run(in_maps, trace=False):
    nc = _build_program()
    return run_bass_kernel_spmd(nc, in_maps, list(range(NCORES)), trace=trace)


def kernel(x, bw1, bb1, bw2, bb2, wc1, bc1, wc2, bc2, wc3, bc3):
    args = [np.asarray(a, np.float32) for a in
            (x, bw1, bb1, bw2, bb2, wc1, bc1, wc2, bc2, wc3, bc3)]
    in_maps = preprocess(*args)
    res = run(in_maps, trace=False)
    y = np.empty((B, OUT), np.float32)
    for c in range(NCORES):
        y[BC * c:BC * (c + 1), :] = res.results[c]["out"].T.astype(np.float32)
    return y


# revision 7
# speedup vs baseline: 2.6792x; 2.6792x over previous
"""Trainium2 Bass kernel for nn_KAN_63230508532179 (dense_mlp).

Model (per reference):
  h = gelu(x[:,:,None] * bw1 + bb1)            # [B,1000,16]
  f = tanh(einsum('bnh,noh->bno', h, bw2)+bb2) # [B,1000,8]
  z = f.reshape(B, 8000)
  z = gelu(z @ wc1.T + bc1)                    # [B,256]
  z = gelu(z @ wc2.T + bc2)                    # [B,128]
  y = z @ wc3.T + bc3                          # [B,300]

Measured reality on this setup: the profiled span is dominated by
host->device input staging (~5.9 GB/s), so the design minimizes wire
bytes.  Strategy: BRANCH-parallel across the 8 cores — core c owns 126
of the 1000 branches over the FULL batch, so the combiner weight wc1
(the largest tensor) is split 8 ways instead of replicated, and the
branch layer-1 weight is never inflated: a per-group 0/1 selector
matrix (built on device with affine_select) broadcasts x rows into the
(j,k) layout and the actual bw1/bb1 are applied as per-partition
scale/bias via a fused DVE tensor_scalar.  Each core accumulates a
partial z1 = f @ wc1_c.T over its branches for all 4096 batch rows;
a ReduceScatter sums partials and hands core c the batch slice
[512c, 512c+512), on which it runs the tiny combiner tail.

Per-core wire bytes ~1.9 MB (vs 11.7 MB for batch-parallel with
replicated weights).  bf16 on the wire for all large tensors; fp16
output.
"""

import os
import sys
from contextlib import ExitStack

sys.path.insert(0, "/opt/trn_rl_repo")
os.environ.setdefault("MYCRO_LOCAL_CACHE", "1")

import numpy as np
import ml_dtypes

import concourse.bass as bass
import concourse.tile as tile
from concourse import bacc, mybir
from concourse.bass_utils import run_bass_kernel_spmd

BF16 = mybir.dt.bfloat16
F32 = mybir.dt.float32
F16 = mybir.dt.float16
NPBF16 = ml_dtypes.bfloat16

B, N, H1, H2 = 4096, 1000, 16, 8
C1, C2, OUT = 256, 128, 300
NCORES = 8
NBR = 126                 # real branch slots per core (last core: 118)
NBP = 128                 # padded branch slots per core
NG = 16                   # groups of 8 branches per core
NPAIR = 8                 # group pairs (= wc1 K-chunks of 128)
BCH = 8                   # batch chunks
BC = 512                  # batch per chunk (= per-core tail batch)

_CACHE = {}


def _build_program():
    if "nc" in _CACHE:
        return _CACHE["nc"]

    nc = bacc.Bacc("TRN2", target_bir_lowering=False, debug=False,
                   num_devices=NCORES)

    xt_d = nc.dram_tensor("xt", [128, B], BF16, kind="ExternalInput")
    sc_d = nc.dram_tensor("sc", [128, NG], F32, kind="ExternalInput")
    sb_d = nc.dram_tensor("sb", [128, NG], F32, kind="ExternalInput")
    w2_d = nc.dram_tensor("w2", [128, NG * 64], BF16, kind="ExternalInput")
    b2_d = nc.dram_tensor("b2", [128, NPAIR], F32, kind="ExternalInput")
    wc1_d = nc.dram_tensor("wc1", [128, NPAIR * C1], BF16, kind="ExternalInput")
    wc2_d = nc.dram_tensor("wc2", [128, 256], BF16, kind="ExternalInput")
    bc1_d = nc.dram_tensor("bc1", [128, 2], F32, kind="ExternalInput")
    bc2_d = nc.dram_tensor("bc2", [128, 1], F32, kind="ExternalInput")
    wc3_d = nc.dram_tensor("wc3", [128, OUT], BF16, kind="ExternalInput")
    bc3_d = nc.dram_tensor("bc3", [128, 3], F32, kind="ExternalInput")
    out_d = nc.dram_tensor("out", [OUT, BC], F16, kind="ExternalOutput")

    AF = mybir.ActivationFunctionType
    ALU = mybir.AluOpType

    with ExitStack() as ctx:
        tc = ctx.enter_context(tile.TileContext(nc))
        consts = ctx.enter_context(tc.tile_pool(name="consts", bufs=1))
        sel_p = ctx.enter_context(tc.tile_pool(name="selp", bufs=2))
        h_pool = ctx.enter_context(tc.tile_pool(name="h", bufs=3))
        g_pool = ctx.enter_context(tc.tile_pool(name="g", bufs=3))
        f_pool = ctx.enter_context(tc.tile_pool(name="f", bufs=3))
        z_pool = ctx.enter_context(tc.tile_pool(name="z", bufs=2))
        ps_h = ctx.enter_context(tc.tile_pool(name="psh", bufs=2, space="PSUM"))
        ps_f = ctx.enter_context(tc.tile_pool(name="psf", bufs=2, space="PSUM"))
        ps_z = ctx.enter_context(tc.tile_pool(name="psz", bufs=2, space="PSUM"))
        dram = ctx.enter_context(tc.tile_pool(name="dram", bufs=1, space="DRAM"))

        def load(d, shape, dt, tag):
            s = consts.tile(shape, dt, tag=tag)
            nc.sync.dma_start(out=s[:], in_=d[:, :])
            return s

        xt_sb = load(xt_d, [128, B], BF16, "xt")
        sc_sb = load(sc_d, [128, NG], F32, "sc")
        sbb_sb = load(sb_d, [128, NG], F32, "sb")
        w2_sb = load(w2_d, [128, NG * 64], BF16, "w2")
        b2_sb = load(b2_d, [128, NPAIR], F32, "b2")
        wc1_sb = load(wc1_d, [128, NPAIR * C1], BF16, "wc1")
        wc2_sb = load(wc2_d, [128, 256], BF16, "wc2")
        bc1_sb = load(bc1_d, [128, 2], F32, "bc1")
        bc2_sb = load(bc2_d, [128, 1], F32, "bc2")
        wc3_sb = load(wc3_d, [128, OUT], BF16, "wc3")
        bc3_sb = load(bc3_d, [128, 3], F32, "bc3")

        # Selector matrices S_u[p, m] = 1 iff 0 <= m - 16*(p - 8u) < 16,
        # i.e. matmul(S_u.T @ x) broadcasts x row 8u+j to h rows 16j..16j+15.
        sel = consts.tile([128, NG * 128], BF16, tag="sel")
        ones = consts.tile([128, 128], BF16, tag="ones")
        nc.gpsimd.memset(ones[:], 1.0)
        for u in range(NG):
            tmp = sel_p.tile([128, 128], BF16)
            # keep where m - 16p + 128u >= 0
            nc.gpsimd.affine_select(
                out=tmp[:], in_=ones[:], pattern=[[1, 128]],
                compare_op=ALU.is_ge, fill=0.0,
                base=128 * u, channel_multiplier=-16)
            # keep where 15 + 16p - 128u - m >= 0
            nc.gpsimd.affine_select(
                out=sel[:, 128 * u:128 * (u + 1)], in_=tmp[:],
                pattern=[[-1, 128]], compare_op=ALU.is_ge, fill=0.0,
                base=15 - 128 * u, channel_multiplier=16)

        z1_part = dram.tile([BCH * C1, BC], F32, tag="z1p")   # [2048, 512]
        z1_my = dram.tile([C1, BC], F32, tag="z1m")

        # ---- main loop: batch chunks x group pairs ----
        for nb in range(BCH):
            z_ps = ps_z.tile([128, 1024], F32)
            for t in range(NPAIR):
                hg = h_pool.tile([128, 1024], BF16)
                for half in range(2):
                    g = 2 * t + half
                    h_ps = ps_h.tile([128, 512], F32)
                    nc.tensor.matmul(
                        h_ps[:], lhsT=sel[:, 128 * g:128 * (g + 1)],
                        rhs=xt_sb[:, BC * nb:BC * (nb + 1)],
                        start=True, stop=True)
                    nc.vector.tensor_scalar(
                        out=hg[:, 512 * half:512 * (half + 1)], in0=h_ps[:],
                        scalar1=sc_sb[:, g:g + 1], scalar2=sbb_sb[:, g:g + 1],
                        op0=ALU.mult, op1=ALU.add)
                hG = g_pool.tile([128, 1024], BF16)
                nc.scalar.activation(hG[:], hg[:], AF.Gelu)
                f_ps = ps_f.tile([128, 512], F32)
                for half in range(2):
                    g = 2 * t + half
                    nc.tensor.matmul(
                        f_ps[64 * half:64 * (half + 1), :],
                        lhsT=w2_sb[:, 64 * g:64 * (g + 1)],
                        rhs=hG[:, 512 * half:512 * (half + 1)],
                        start=True, stop=True)
                fT = f_pool.tile([128, 512], BF16)
                nc.scalar.activation(fT[:], f_ps[:], AF.Tanh,
                                     bias=b2_sb[:, t:t + 1], scale=1.0)
                last = t == NPAIR - 1
                nc.tensor.matmul(z_ps[:, 0:512],
                                 lhsT=wc1_sb[:, 256 * t:256 * t + 128],
                                 rhs=fT[:], start=(t == 0), stop=last,
                                 skip_group_check=True)
                nc.tensor.matmul(z_ps[:, 512:1024],
                                 lhsT=wc1_sb[:, 256 * t + 128:256 * t + 256],
                                 rhs=fT[:], start=(t == 0), stop=last,
                                 skip_group_check=True)
            z_sb = z_pool.tile([128, 1024], F32, tag="z_sb")
            nc.vector.tensor_copy(z_sb[:], z_ps[:])
            nc.sync.dma_start(out=z1_part[256 * nb:256 * nb + 128, :],
                              in_=z_sb[:, 0:512])
            nc.sync.dma_start(out=z1_part[256 * nb + 128:256 * nb + 256, :],
                              in_=z_sb[:, 512:1024])

        # ---- cross-core reduction: core c gets batch chunk c, summed ----
        nc.gpsimd.collective_compute(
            "ReduceScatter", ALU.add,
            replica_groups=[list(range(NCORES))],
            ins=[z1_part.opt()], outs=[z1_my.opt()],
        )

        # ---- combiner tail on this core's 512-row batch slice ----
        z1g = []
        for hk in range(2):
            z1f = z_pool.tile([128, 512], F32, tag="z1f")
            nc.sync.dma_start(out=z1f[:],
                              in_=z1_my[128 * hk:128 * (hk + 1), :])
            zg = z_pool.tile([128, 512], BF16, tag="z1g")
            nc.scalar.activation(zg[:], z1f[:], AF.Gelu,
                                 bias=bc1_sb[:, hk:hk + 1], scale=1.0)
            z1g.append(zg)

        z2_ps = ps_h.tile([128, 512], F32, tag="h_ps")
        nc.tensor.matmul(z2_ps[:], lhsT=wc2_sb[:, 0:128], rhs=z1g[0][:],
                         start=True, stop=False, skip_group_check=True)
        nc.tensor.matmul(z2_ps[:], lhsT=wc2_sb[:, 128:256], rhs=z1g[1][:],
                         start=False, stop=True, skip_group_check=True)
        z2 = z_pool.tile([128, 512], BF16, tag="z2")
        nc.scalar.activation(z2[:], z2_ps[:], AF.Gelu,
                             bias=bc2_sb[:, 0:1], scale=1.0)

        for i, mrows in ((0, 128), (1, 128), (2, 44)):
            o_ps = ps_f.tile([128, 512], F32, tag="f_ps")
            nc.tensor.matmul(o_ps[0:mrows, :],
                             lhsT=wc3_sb[:, 128 * i:128 * i + mrows],
                             rhs=z2[:], start=True, stop=True)
            o_sb = z_pool.tile([128, 512], F16, tag="o")
            nc.vector.tensor_scalar_add(o_sb[0:mrows, :], o_ps[0:mrows, :],
                                        bc3_sb[0:mrows, i:i + 1])
            nc.sync.dma_start(out=out_d[128 * i:128 * i + mrows, :],
                              in_=o_sb[0:mrows, :])

    nc.compile()
    _CACHE["nc"] = nc
    return nc


def preprocess(x, bw1, bb1, bw2, bb2, wc1, bc1, wc2, bc2, wc3, bc3):
    """Host-side repack of full inputs into per-core input maps."""
    f32 = np.float32
    NPB = NCORES * NBR            # 1008 padded branches

    bw1p = np.zeros((NPB, H1), f32); bw1p[:N] = bw1
    bb1p = np.zeros((NPB, H1), f32); bb1p[:N] = bb1
    bw2p = np.zeros((NPB, H2, H1), f32); bw2p[:N] = bw2
    bb2p = np.zeros((NPB, H2), f32); bb2p[:N] = bb2
    xp = np.zeros((NPB, B), f32); xp[:N] = np.asarray(x).T

    wc1r = np.asarray(wc1).reshape(C1, N, H2)

    # tail constants (replicated on all cores; small)
    wc2_sb = np.ascontiguousarray(
        wc2.T.reshape(2, 128, C2).transpose(1, 0, 2).reshape(128, 256)
    ).astype(NPBF16)
    bc1_sb = np.ascontiguousarray(bc1.reshape(2, 128).T.astype(f32))
    bc2_sb = np.ascontiguousarray(bc2.reshape(C2, 1).astype(f32))
    wc3_sb = np.ascontiguousarray(wc3.T).astype(NPBF16)
    bc3p = np.zeros(384, f32); bc3p[:OUT] = bc3
    bc3_sb = np.ascontiguousarray(bc3p.reshape(3, 128).T)

    in_maps = []
    for c in range(NCORES):
        n0 = c * NBR
        lw1 = np.zeros((NBP, H1), f32); lw1[:NBR] = bw1p[n0:n0 + NBR]
        lb1 = np.zeros((NBP, H1), f32); lb1[:NBR] = bb1p[n0:n0 + NBR]
        lw2 = np.zeros((NBP, H2, H1), f32); lw2[:NBR] = bw2p[n0:n0 + NBR]
        lb2 = np.zeros((NBP, H2), f32); lb2[:NBR] = bb2p[n0:n0 + NBR]
        lx = np.zeros((NBP, B), f32); lx[:NBR] = xp[n0:n0 + NBR]

        # scale/bias [128, NG]: row 16j+k, col g -> lw1[8g+j, k]
        sc = lw1.reshape(NG, 8, H1).transpose(1, 2, 0).reshape(128, NG)
        sbb = lb1.reshape(NG, 8, H1).transpose(1, 2, 0).reshape(128, NG)

        # W2 block-diagonal [128, NG*64]: [16j+k, 64g+8j'+o] nonzero iff j==j'
        W2 = np.zeros((8, H1, NG, 8, H2), f32)      # [j, k, g, j', o]
        lw2g = lw2.reshape(NG, 8, H2, H1)           # [g, j, o, k]
        for j in range(8):
            W2[j, :, :, j, :] = lw2g[:, j].transpose(2, 0, 1)   # [k, g, o]
        w2_sb = W2.reshape(128, NG * 64).astype(NPBF16)

        # b2 [128, NPAIR]: row 64h+8j+o, col t -> lb2[16t+8h+j, o]
        b2_sb = np.ascontiguousarray(
            lb2.reshape(NPAIR, 2, 8, H2).transpose(1, 2, 3, 0).reshape(128, NPAIR))

        # wc1 K-slice [128, NPAIR*C1]: row 64h+8j+o of chunk t,
        # col 256t+mm -> wc1[mm, 8*(n0+16t+8h+j)+o]; zero rows for pads
        wc1l = np.zeros((NBP, H2, C1), f32)         # [local branch, o, mm]
        gidx = n0 + np.arange(NBP)
        valid = (np.arange(NBP) < NBR) & (gidx < N)
        vi = np.where(valid)[0]
        wc1l[vi] = wc1r[:, gidx[vi], :].transpose(1, 2, 0)
        wc1_sb = np.ascontiguousarray(
            wc1l.reshape(NPAIR, 2, 8, H2, C1)
            .transpose(1, 2, 3, 0, 4).reshape(128, NPAIR * C1)).astype(NPBF16)

        m = {
            "xt": np.ascontiguousarray(lx.astype(NPBF16)),
            "sc": np.ascontiguousarray(sc), "sb": np.ascontiguousarray(sbb),
            "w2": np.ascontiguousarray(w2_sb), "b2": b2_sb, "wc1": wc1_sb,
            "wc2": wc2_sb, "bc1": bc1_sb, "bc2": bc2_sb,
            "wc3": wc3_sb, "bc3": bc3_sb,
        }
        in_maps.append(m)
    return in_maps


def run(in_maps, trace=False):
    nc = _build_program()
    return run_bass_kernel_spmd(nc, in_maps, list(range(NCORES)), trace=trace)


def kernel(x, bw1, bb1, bw2, bb2, wc1, bc1, wc2, bc2, wc3, bc3):
    args = [np.asarray(a, np.float32) for a in
            (x, bw1, bb1, bw2, bb2, wc1, bc1, wc2, bc2, wc3, bc3)]
    in_maps = preprocess(*args)
    res = run(in_maps, trace=False)
    y = np.empty((B, OUT), np.float32)
    for c in range(NCORES):
        y[BC * c:BC * (c + 1), :] = res.results[c]["out"].T.astype(np.float32)
    return y
